# revision 22
# baseline (speedup 1.0000x reference)
"""Trainium2 Bass kernel for nn_Decoder (MusicVAE-style hierarchical LSTM decoder).

v2 strategy (8-way model parallelism over the 4096-wide gate dim):
- Every gate matmul is split into two N=256 halves written to PSUM
  partition ranges [0:64] (A) and [64:128] (B). The PE runs A/B pairs
  concurrently in distinct column groups (2x streaming bandwidth).
- Gate columns are packed [i|f|o|g] x 64 per half, where A carries the
  lo 64 dims and B the hi 64 dims of the core's 128-dim gate slice. All
  LSTM pointwise ops then run on full-lane [128, 64] tiles, the cell
  state lives as [128, 64] (batch x lo | batch x hi), and h is born in a
  layout where two [64,64] PE transposes produce the [dims, batch]
  staging tile directly.
- All matmul operands are bf16 (PSUM accumulation fp32, cell state
  fp32); broadcasts move half the bytes. Numpy-emulated rel err 4.7e-3.
- g-gate uses tanh directly (no sigmoid-doubling), one sigmoid covers
  i|f|o of both halves' batch rows, tmp_i/tmp_f fuse into one DVE op.
- PE stream is reordered so transposes/proj never stall the next step's
  E/h matmuls; latent@Wxc0+bc0 is hoisted out of the conductor loop.
- Hidden-state exchange per step via remote_dma_broadcast (SBUF->SBUF,
  all 8 cores); conductor embeddings DRAM-bounced per subsequence.
"""

import os
import sys

for _p in ("/opt/trn_rl_repo", "/root/.axon_site/_ro/trn_rl_repo"):
    if os.path.isdir(_p) and _p not in sys.path:
        sys.path.insert(0, _p)
        break

import numpy as np

from concourse import bass, mybir, bacc

F32 = mybir.dt.float32
BF16 = mybir.dt.bfloat16

NC = 8           # cores
B = 64           # batch
H = 1024         # decoder hidden
GSL = 512        # per-core gate slice (4*H/NC)
HGS = 256        # half gate slice
KT_H = 8         # K tiles of 128 over H
KT_L = 4         # K tiles of 128 over LATENT
KT_X = 7         # K tiles of 64 over the note input (512-pad, tile 7 all-zero)
INPUT = 389
INPUT_PAD = 512
SL = 64          # slot width (columns) in gathered buffers

RD = [(0, k) for k in range(NC)]


def build(nsub, nnotes, full_out=True):
    CT = nsub
    T = nsub * nnotes
    TOUT = T if full_out else 1
    nc = bacc.Bacc(num_devices=NC)

    # ---------------- DRAM parameters (per-core data) ----------------
    dp = nc.declare_dram_parameter
    latT_d = dp("latT", [128, KT_L * SL], BF16, isOutput=False)
    h0T_d = dp("h0T", [nsub, 128, 2 * KT_H * SL], BF16, isOutput=False)
    c0s_d = dp("c0s", [nsub * 2, 128, SL], F32, isOutput=False)
    wx0_d = dp("wx0", [64, KT_X * GSL], BF16, isOutput=False)
    wh0_d = dp("wh0", [128, KT_H * GSL], BF16, isOutput=False)
    wx1_d = dp("wx1", [128, KT_H * GSL], BF16, isOutput=False)
    wh1_d = dp("wh1", [128, KT_H * GSL], BF16, isOutput=False)
    wdoT_d = dp("wdoT", [128, KT_H * SL], BF16, isOutput=False)
    wemb_d = dp("wemb", [64, KT_H * GSL], BF16, isOutput=False)
    wxc0_d = dp("wxc0", [128, KT_L * GSL], BF16, isOutput=False)
    whc0_d = dp("whc0", [128, KT_H * GSL], BF16, isOutput=False)
    wxc1_d = dp("wxc1", [128, KT_H * GSL], BF16, isOutput=False)
    whc1_d = dp("whc1", [128, KT_H * GSL], BF16, isOutput=False)
    wcoT_d = dp("wcoT", [128, KT_H * SL], BF16, isOutput=False)
    b1_d = dp("b1r", [1, GSL], BF16, isOutput=False)
    bdo_d = dp("bdoc", [1, SL], BF16, isOutput=False)
    b0_d = dp("b0r", [1, GSL], BF16, isOutput=False)
    bc0_d = dp("bc0r", [1, GSL], BF16, isOutput=False)
    bc1_d = dp("bc1r", [1, GSL], BF16, isOutput=False)
    bco_d = dp("bcoc", [1, SL], BF16, isOutput=False)
    ones_d = dp("onesr", [1, SL], BF16, isOutput=False)
    id64_d = dp("id64", [64, 64], BF16, isOutput=False)
    idT_d = dp("idT", [128, 128], BF16, isOutput=False)
    out_d = dp("out", [TOUT, 64, 32], F32, isOutput=True)  # bf16 pairs
    est_d = nc.dram_tensor("est", [nsub, 128, HGS], BF16)

    import contextlib
    with contextlib.ExitStack() as ctx:
        e = ctx.enter_context
        sb = lambda name, shape, dt=F32: e(nc.sbuf_tensor(name, shape, dt))
        ps = lambda name, shape: e(nc.psum_tensor(name, shape, F32))
        sem = lambda name: e(nc.semaphore(name))

        # weights / constants in SBUF
        LAT = sb("LAT", [128, KT_L * SL], BF16)
        LATC = sb("LATC", [128, HGS], BF16)    # bc0 + latent@Wxc0 (both halves)
        H0T = sb("H0T", [128, 2 * (2 * KT_H * SL)], BF16)   # ping-pong per subseq
        WX0 = sb("WX0", [64, KT_X * GSL], BF16)
        WH0 = sb("WH0", [128, KT_H * GSL], BF16)
        WX1 = sb("WX1", [128, KT_H * GSL], BF16)
        WH1 = sb("WH1", [128, KT_H * GSL], BF16)
        WDOT = sb("WDOT", [128, KT_H * SL], BF16)
        WEMB = sb("WEMB", [64, KT_H * GSL], BF16)
        WXC0 = sb("WXC0", [128, KT_L * GSL], BF16)
        WHC0 = sb("WHC0", [128, KT_H * GSL], BF16)
        WXC1 = sb("WXC1", [128, KT_H * GSL], BF16)
        WHC1 = sb("WHC1", [128, KT_H * GSL], BF16)
        WCOT = sb("WCOT", [128, KT_H * SL], BF16)
        B1 = sb("B1", [1, GSL], BF16)
        BDO = sb("BDO", [1, SL], BF16)
        B0 = sb("B0", [1, GSL], BF16)
        BC0 = sb("BC0", [1, GSL], BF16)
        BC1 = sb("BC1", [1, GSL], BF16)
        BCO = sb("BCO", [1, SL], BF16)
        ONES = sb("ONES", [1, SL], BF16)
        ID64 = sb("ID64", [64, 64], BF16)
        IDT = sb("IDT", [128, 128], BF16)
        EBUF = sb("EBUF", [128, 2 * HGS], BF16)
        EDEC = sb("EDEC", [128, 2 * HGS], BF16)

        # gathered state buffers (ping-pong x2), [dims(128), 8 slots x 64]
        HD0 = [sb(f"HD0_{i}", [128, NC * SL], BF16) for i in range(2)]
        HD1 = [sb(f"HD1_{i}", [128, NC * SL], BF16) for i in range(2)]
        NT = [sb(f"NT_{i}", [128, NC * SL], BF16) for i in range(2)]
        HC0 = [sb(f"HC0_{i}", [128, NC * SL], BF16) for i in range(2)]
        HC1 = [sb(f"HC1_{i}", [128, NC * SL], BF16) for i in range(2)]
        EMBT = [sb(f"EMBT_{i}", [128, NC * SL], BF16) for i in range(2)]

        # staging for outgoing tiles
        HSTG0 = [sb(f"HSTG0_{i}", [128, SL], BF16) for i in range(2)]
        HSTG1 = [sb(f"HSTG1_{i}", [128, SL], BF16) for i in range(2)]
        SNT = [sb(f"SNT_{i}", [128, SL], BF16) for i in range(2)]
        SEM_ = [sb(f"SEM_{i}", [128, SL], BF16) for i in range(2)]

        # pointwise tiles: rows 0:64 = batch x lo-dims, 64:128 = batch x hi
        S0 = sb("S0", [128, 192])     # [sig_i | sig_f | sig_o]
        S1 = sb("S1", [128, 192])
        GCD0 = sb("GCD0", [128, 128])  # [tanh_g | c] decoder l0
        GCD1 = sb("GCD1", [128, 128])
        GCC0 = sb("GCC0", [128, 128])  # conductor
        GCC1 = sb("GCC1", [128, 128])
        TMP0 = sb("TMP0", [128, 128])  # [tmp_i | tmp_f]
        TMP1 = sb("TMP1", [128, 128])
        TT0 = sb("TT0", [128, SL])     # tanh(c)
        TT1 = sb("TT1", [128, SL])
        HT0 = sb("HT0", [128, SL], BF16)   # h (pre-transpose)
        HT1 = sb("HT1", [128, SL], BF16)

        # psum — full-bank [128, 512] tensors (2KB/partition) so that the
        # per-partition zero-region semantics of start_tensor_calc stay
        # aligned; only cols 0:HGS are used by the half-split groups.
        psd0 = [ps(f"psd0_{i}", [128, 512]) for i in range(2)]
        psd1 = [ps(f"psd1_{i}", [128, 512]) for i in range(2)]
        psem = ps("psem", [128, 512])
        pspr = ps("pspr", [128, 512])  # proj uses [0:64, 0:64]
        pstr0 = ps("pstr0", [128, 512])  # bf16 [128, 64] used via bitcast
        pstr1 = ps("pstr1", [128, 512])

        # semaphores
        dw = sem("dw"); dh = sem("dh"); dgc = sem("dgc"); gi = sem("gi")
        lat_s = sem("lat_s")
        do = [sem("doa"), sem("dob")]
        de = [sem("dea"), sem("deb")]; ep = [sem("epa"), sem("epb")]
        pe_s = sem("pe_s"); act_s = sem("act_s"); dve_s = sem("dve_s")
        r_h0 = sem("r_h0"); r_h1 = sem("r_h1"); r_nt = sem("r_nt"); r_em = sem("r_em")
        l_h0 = [sem("l_h0a"), sem("l_h0b")]; l_h1 = [sem("l_h1a"), sem("l_h1b")]
        l_nt = [sem("l_nta"), sem("l_ntb")]; l_em = [sem("l_ema"), sem("l_emb")]
        prep = sem("prep")

        N_MEMSET = 12
        N_WLOAD = 21

        # ---- sem threshold helpers ----
        def pe_c(ct, k):      # d0c=1, tr0=2, d1c=3, tr1=4, em=5, E=6
            return 6 * ct + k

        def pe_d(t, k):       # d0=1, tr0=2, d1=3, tr1=4, pr=5
            return 6 * CT + 5 * t + k

        def act_c(ct, k):     # tg0=1 sig0=2 tc0=3 tg1=4 sig1=5 tc1=6 temb=7
            return 7 * ct + k

        def act_d(t, k):      # tg0=1 sig0=2 tc0=3 tg1=4 sig1=5 tc1=6 tnote=7
            return 7 * CT + 7 * t + k

        def act_prev(t, k):   # step t-1's act event (conductor for t=0)
            return act_d(t - 1, k) if t >= 1 else act_c(CT - 1, k)

        def dve_c(ct, k):     # bm0=1 cn0=2 h0=3 cp0=4 bm1=5 cn1=6 h1=7 cp1=8 E=9
            return 9 * ct + k

        def dve_d(t, k):      # same minus E: 8/step
            return 9 * CT + 8 * t + k

        def dve_prev(t, k):
            return dve_d(t - 1, k) if t >= 1 else dve_c(CT - 1, k)

        def snd_c(ct):        # parity-(ct%2) sends strictly before conductor ct
            return (ct - ct % 2) // 2

        def snd_d(t):
            p = t % 2
            return (CT - p + 1) // 2 + (t - p) // 2

        IDB = IDT[64:128, 64:128]   # identity block at partitions 64:128

        with nc.Block() as block:

            # ================= SYNC: DMAs =================
            @block.sync
            def _(sy):
                loads = [
                    (LAT, latT_d),
                    (WX0, wx0_d), (WH0, wh0_d), (WX1, wx1_d), (WH1, wh1_d),
                    (WDOT, wdoT_d), (WEMB, wemb_d),
                    (WXC0, wxc0_d), (WHC0, whc0_d), (WXC1, wxc1_d), (WHC1, whc1_d),
                    (WCOT, wcoT_d),
                    (B1, b1_d), (BDO, bdo_d), (B0, b0_d),
                    (BC0, bc0_d), (BC1, bc1_d), (BCO, bco_d),
                    (ONES, ones_d), (ID64, id64_d), (IDT, idT_d),
                ]
                for dst, src in loads:
                    sy.dma_start(out=dst[:, :], in_=src[:, :]).then_inc(dw, 16)
                # first subsequence h/c init
                sy.dma_start(out=H0T[:, 0:2 * KT_H * SL], in_=h0T_d[0, :, :]).then_inc(dh, 16)
                sy.dma_start(out=GCD0[:, 64:128], in_=c0s_d[0, :, :]).then_inc(dgc, 16)
                sy.dma_start(out=GCD1[:, 64:128], in_=c0s_d[1, :, :]).then_inc(dgc, 16)

                # conductor: store E_s to DRAM scratch
                for ct in range(CT):
                    sy.wait_ge(dve_s, dve_c(ct, 9))
                    sy.dma_start(out=est_d[ct],
                                 in_=EBUF[:, HGS * (ct % 2):HGS * (ct % 2 + 1)]
                                 ).then_inc(de[ct % 2], 16)

                def n_stores(par):
                    return len([c for c in range(CT) if c % 2 == par])

                # first E prefetch (s=0)
                sy.wait_ge(de[0], 16 * n_stores(0))
                sy.dma_start(out=EDEC[:, 0:HGS], in_=est_d[0]).then_inc(ep[0], 16)

                # decoder phase: per-subsequence prefetch + output DMA
                for t in range(T):
                    s, n = divmod(t, nnotes)
                    if n == 2 and s + 1 < nsub:
                        sy.wait_ge(pe_s, pe_d(t - 1, 5))
                        sp = (s + 1) % 2
                        sy.dma_start(
                            out=H0T[:, sp * (2 * KT_H * SL):(sp + 1) * (2 * KT_H * SL)],
                            in_=h0T_d[s + 1, :, :],
                        ).then_inc(dh, 16)
                        sy.wait_ge(de[sp], 16 * n_stores(sp))
                        sy.dma_start(out=EDEC[:, sp * HGS:(sp + 1) * HGS],
                                     in_=est_d[s + 1]).then_inc(ep[sp], 16)
                    if n == nnotes - 1 and s + 1 < nsub:
                        # cell-state init for next subseq, after tanh_c1(t) read c
                        sy.wait_ge(act_s, act_d(t, 6))
                        sy.dma_start(out=GCD0[:, 64:128],
                                     in_=c0s_d[(s + 1) * 2, :, :]).then_inc(dgc, 16)
                        sy.dma_start(out=GCD1[:, 64:128],
                                     in_=c0s_d[(s + 1) * 2 + 1, :, :]).then_inc(dgc, 16)
                    p = t % 2
                    sy.wait_ge(act_s, act_d(t, 7))
                    sy.dma_start(out=out_d[t if full_out else 0],
                                 in_=SNT[p][0:64, :].bitcast(F32)).then_inc(do[p], 16)

            # ================= GPSIMD: memsets + exchanges =================
            @block.gpsimd
            def _(g):
                U32 = mybir.dt.uint32
                for tile in (NT[0], NT[1], EMBT[0], EMBT[1]):
                    g.memset(tile[:, :].bitcast(U32), 0).then_inc(gi, 1)
                for tile in (SNT[0], SNT[1], SEM_[0], SEM_[1], HC0[1], HC1[1]):
                    g.memset(tile[:, :].bitcast(U32), 0).then_inc(gi, 1)
                g.memset(GCC0[:, 64:128].bitcast(U32), 0).then_inc(gi, 1)
                g.memset(GCC1[:, 64:128].bitcast(U32), 0).then_inc(gi, 1)
                g.wait_ge(gi, N_MEMSET)
                pid = g.partition_id()
                off = g.scalar_reg_alu(mybir.AluOpType.mult, pid, SL)
                np_ = [0]

                def step_bcasts(specs):
                    # prep all descriptors first, then trigger in FIFO order
                    for stg, gath, rsem, lsem, _, _ in specs:
                        g.remote_dma_broadcast(
                            out_ap=gath[:, bass.ds(off, SL)], in_ap=stg[:, :],
                            remote_sem=rsem, local_sem=lsem, rdests=RD,
                        ).then_inc(prep, 1)
                        np_[0] += 1
                    g.wait_ge(prep, np_[0])
                    for _, _, _, _, wait_sem, wait_val in specs:
                        g.wait_ge(wait_sem, wait_val)
                        g.trigger_dma(count=1)

                for ct in range(CT):
                    p = ct % 2
                    step_bcasts([
                        (HSTG0[p], HC0[p], r_h0, l_h0[p], dve_s, dve_c(ct, 4)),
                        (HSTG1[p], HC1[p], r_h1, l_h1[p], dve_s, dve_c(ct, 8)),
                        (SEM_[p], EMBT[p], r_em, l_em[p], act_s, act_c(ct, 7)),
                    ])
                for t in range(T):
                    p = t % 2
                    step_bcasts([
                        (HSTG0[p], HD0[p], r_h0, l_h0[p], dve_s, dve_d(t, 4)),
                        (HSTG1[p], HD1[p], r_h1, l_h1[p], dve_s, dve_d(t, 8)),
                        (SNT[p], NT[p], r_nt, l_nt[p], act_s, act_d(t, 7)),
                    ])

            # ================= TENSOR: matmuls + transposes =================
            @block.tensor
            def _(t_):
                def mmh(ps_, lhsA, lhsB, rhs, off, first, last, inc=False):
                    """One logical k-tile: A/B half pair (concurrent col groups).
                    skip_group_check: the sim's group tracker mis-addresses
                    psum APs with base partition 64 (data path is correct)."""
                    t_.matmul(ps_[0:64, 0:HGS], lhsA, rhs[:, off:off + HGS],
                              start=first, stop=last, skip_group_check=True)
                    m = t_.matmul(ps_[64:128, 0:HGS], lhsB, rhs[:, off + HGS:off + GSL],
                                  start=first, stop=last, skip_group_check=True)
                    if inc:
                        m.then_inc(pe_s, 1)
                    return m

                def gates(ps_, ktiles, first=True, last=True, inc=False):
                    """ktiles: list of (lhsT, rhs, col_off). lhsT spans both halves."""
                    nk = len(ktiles)
                    for j, (lh, rhs, off) in enumerate(ktiles):
                        mmh(ps_, lh, lh, rhs, off,
                            first and j == 0, last and j == nk - 1,
                            inc and j == nk - 1)

                def bias2(ps_, brow, first):
                    t_.matmul(ps_[0:64, 0:HGS], ONES[:, :], brow[:, 0:HGS],
                              start=first, stop=False, skip_group_check=True)
                    t_.matmul(ps_[64:128, 0:HGS], ONES[:, :], brow[:, HGS:GSL],
                              start=first, stop=False, skip_group_check=True)

                def ident2(ps_, src, off, first):
                    t_.matmul(ps_[0:64, 0:HGS], ID64[:, :], src[0:64, off:off + HGS],
                              start=first, stop=False, skip_group_check=True)
                    t_.matmul(ps_[64:128, 0:HGS], IDB, src[64:128, off:off + HGS],
                              start=first, stop=False, skip_group_check=True)

                def transp(pstr, HT):
                    t_.matmul(pstr[:, :].bitcast(BF16)[0:64, 0:64],
                              HT[0:64, :], ID64[:, :], is_transpose=True,
                              skip_group_check=True)
                    return t_.matmul(pstr[:, :].bitcast(BF16)[64:128, 0:64],
                                     HT[64:128, :], IDB, is_transpose=True,
                                     skip_group_check=True)

                t_.wait_ge(dw, 16 * N_WLOAD)
                t_.wait_ge(gi, N_MEMSET)

                # ---- prologue: LATC = bc0 + latent @ Wxc0 (into psd0[0]) ----
                bias2(psd0[0], BC0, True)
                for k in range(KT_L):
                    m = mmh(psd0[0], LAT[:, SL * k:SL * (k + 1)],
                            LAT[:, SL * k:SL * (k + 1)],
                            WXC0, GSL * k, False, k == KT_L - 1)
                m.then_inc(lat_s, 1)

                # ---------- conductor ----------
                for ct in range(CT):
                    p, p1 = ct % 2, (ct - 1) % 2
                    # layer c0 gates: LATC identity + hc0(ct-1) @ Whc0
                    if ct >= 2:
                        t_.wait_ge(act_s, act_c(ct - 2, 2))
                    else:
                        t_.wait_ge(lat_s, 2)
                    ident2(psd0[p], LATC, 0, True)
                    if ct >= 1:
                        t_.wait_ge(r_h0, 16 * ct)
                    gates(psd0[p],
                          [(HC0[p1][:, SL * k:SL * (k + 1)], WHC0, GSL * k)
                           for k in range(KT_H)],
                          first=False, last=True, inc=True)       # pe_c(ct,1)
                    # transpose hc0
                    t_.wait_ge(dve_s, dve_c(ct, 3))
                    transp(pstr0, HT0).then_inc(pe_s, 1)          # pe_c(ct,2)
                    # layer c1 gates
                    if ct >= 2:
                        t_.wait_ge(act_s, act_c(ct - 2, 5))
                    else:
                        t_.wait_ge(lat_s, 2)
                    bias2(psd1[p], BC1, True)
                    if ct >= 1:
                        t_.wait_ge(r_h1, 16 * ct)
                    gates(psd1[p],
                          [(HC1[p1][:, SL * k:SL * (k + 1)], WHC1, GSL * k)
                           for k in range(KT_H)],
                          first=False, last=False)
                    t_.wait_ge(r_h0, 16 * (ct + 1))
                    gates(psd1[p],
                          [(HC0[p][:, SL * k:SL * (k + 1)], WXC1, GSL * k)
                           for k in range(KT_H)],
                          first=False, last=True, inc=True)       # pe_c(ct,3)
                    t_.wait_ge(dve_s, dve_c(ct, 7))
                    transp(pstr1, HT1).then_inc(pe_s, 1)          # pe_c(ct,4)
                    # emb projection (transposed, sharded)
                    if ct >= 1:
                        t_.wait_ge(act_s, act_c(ct - 1, 7))
                    t_.matmul(pspr[0:64, 0:64], BCO[:, :], ONES[:, :], start=True, stop=False)
                    t_.wait_ge(r_h1, 16 * (ct + 1))
                    for k in range(KT_H):
                        m = t_.matmul(pspr[0:64, 0:64], WCOT[:, SL * k:SL * (k + 1)],
                                      HC1[p][:, SL * k:SL * (k + 1)],
                                      start=False, stop=k == KT_H - 1)
                    m.then_inc(pe_s, 1)                            # pe_c(ct,5)
                    # E_s = b0 + emb@Wemb (gathered EMBT)
                    if ct >= 1:
                        t_.wait_ge(dve_s, dve_c(ct - 1, 9))
                    bias2(psem, B0, True)
                    t_.wait_ge(r_em, 16 * (ct + 1))
                    for j in range(KT_H):
                        mmh(psem, EMBT[p][0:64, SL * j:SL * (j + 1)],
                            EMBT[p][0:64, SL * j:SL * (j + 1)],
                            WEMB, GSL * j, False, j == KT_H - 1, inc=j == KT_H - 1)
                    # pe_c(ct,6)

                # ---------- decoder ----------
                # d0-part1(0) and d1-part1(0): issued in prologue position
                def d0_part1(t):
                    s, n = divmod(t, nnotes)
                    sb_ = s % 2
                    h0base = sb_ * (2 * KT_H * SL)
                    t_.wait_ge(act_s, act_prev(t - 1, 2) if t >= 1 else act_c(CT - 2, 2))
                    if n == 0:
                        t_.wait_ge(ep[s % 2], 16 * ((s - s % 2) // 2 + 1))
                    ident2(psd0[t % 2], EDEC, (s % 2) * HGS, True)
                    if n == 0:
                        t_.wait_ge(dh, 16 * (s + 1))
                        stat = lambda k: H0T[:, h0base + SL * k:h0base + SL * (k + 1)]
                    else:
                        stat = lambda k: HD0[(t - 1) % 2][:, SL * k:SL * (k + 1)]
                    gates(psd0[t % 2],
                          [(stat(k), WH0, GSL * k) for k in range(KT_H)],
                          first=False, last=False)

                def d1_part1(t):
                    s, n = divmod(t, nnotes)
                    sb_ = s % 2
                    h0base = sb_ * (2 * KT_H * SL)
                    t_.wait_ge(act_s, act_prev(t - 1, 5) if t >= 1 else act_c(CT - 2, 5))
                    bias2(psd1[t % 2], B1, True)
                    if n == 0:
                        stat1 = lambda k: H0T[:, h0base + (KT_H + k) * SL:h0base + (KT_H + k + 1) * SL]
                    else:
                        stat1 = lambda k: HD1[(t - 1) % 2][:, SL * k:SL * (k + 1)]
                    gates(psd1[t % 2],
                          [(stat1(k), WH1, GSL * k) for k in range(KT_H)],
                          first=False, last=False)

                d0_part1(0)
                d1_part1(0)

                for t in range(T):
                    p = t % 2
                    s, n = divmod(t, nnotes)
                    # ---- d0 x-part (notes) ----
                    if t >= 1:
                        t_.wait_ge(r_nt, 16 * t)
                    for j in range(KT_X):
                        mmh(psd0[p], NT[(t - 1) % 2][0:64, SL * j:SL * (j + 1)],
                            NT[(t - 1) % 2][0:64, SL * j:SL * (j + 1)],
                            WX0, GSL * j, False, j == KT_X - 1, inc=j == KT_X - 1)
                    # pe_d(t,1)
                    t_.wait_ge(dve_s, dve_d(t, 3))
                    transp(pstr0, HT0).then_inc(pe_s, 1)           # pe_d(t,2)
                    # ---- d1 x-part (h0 this step) ----
                    t_.wait_ge(r_h0, 16 * (CT + t + 1))
                    gates(psd1[p],
                          [(HD0[p][:, SL * k:SL * (k + 1)], WX1, GSL * k)
                           for k in range(KT_H)],
                          first=False, last=True, inc=True)        # pe_d(t,3)
                    t_.wait_ge(dve_s, dve_d(t, 7))
                    transp(pstr1, HT1).then_inc(pe_s, 1)           # pe_d(t,4)
                    # ---- d0-part1 for t+1 (fills h1 pointwise window) ----
                    if t + 1 < T:
                        d0_part1(t + 1)
                    # ---- note projection ----
                    t_.wait_ge(act_s, act_prev(t, 7))
                    t_.matmul(pspr[0:64, 0:64], BDO[:, :], ONES[:, :], start=True, stop=False)
                    t_.wait_ge(r_h1, 16 * (CT + t + 1))
                    for k in range(KT_H):
                        m = t_.matmul(pspr[0:64, 0:64], WDOT[:, SL * k:SL * (k + 1)],
                                      HD1[p][:, SL * k:SL * (k + 1)],
                                      start=False, stop=k == KT_H - 1)
                    m.then_inc(pe_s, 1)                            # pe_d(t,5)
                    # ---- d1-part1 for t+1 ----
                    if t + 1 < T:
                        d1_part1(t + 1)

            # ================= SCALAR (ACT) =================
            @block.scalar
            def _(a):
                SIG = mybir.ActivationFunctionType.Sigmoid
                TANH = mybir.ActivationFunctionType.Tanh

                def layer_acts(pe_done, dve_guard, dve_cn, S, GC, TTt, psrc):
                    a.wait_ge(pe_s, pe_done)
                    if dve_guard is not None:
                        a.wait_ge(dve_s, dve_guard)
                    a.activation(GC[:, 0:64], psrc[:, 192:256], TANH).then_inc(act_s, 1)
                    a.activation(S[:, :], psrc[:, 0:192], SIG).then_inc(act_s, 1)
                    a.wait_ge(dve_s, dve_cn)
                    a.activation(TTt[:, :], GC[:, 64:128], TANH).then_inc(act_s, 1)

                # conductor
                for ct in range(CT):
                    p = ct % 2
                    layer_acts(pe_c(ct, 1), dve_c(ct - 1, 3) if ct >= 1 else None,
                               dve_c(ct, 2), S0, GCC0, TT0, psd0[p])
                    layer_acts(pe_c(ct, 3), dve_c(ct - 1, 7) if ct >= 1 else None,
                               dve_c(ct, 6), S1, GCC1, TT1, psd1[p])
                    a.wait_ge(pe_s, pe_c(ct, 5))
                    if snd_c(ct) > 0:
                        a.wait_ge(l_em[ct % 2], 16 * snd_c(ct))
                    a.activation(SEM_[p][0:64, :], pspr[0:64, 0:64], TANH).then_inc(act_s, 1)
                # decoder
                for t in range(T):
                    p = t % 2
                    layer_acts(pe_d(t, 1), dve_prev(t, 3), dve_d(t, 2),
                               S0, GCD0, TT0, psd0[p])
                    layer_acts(pe_d(t, 3), dve_prev(t, 7), dve_d(t, 6),
                               S1, GCD1, TT1, psd1[p])
                    a.wait_ge(pe_s, pe_d(t, 5))
                    if t >= 2:
                        a.wait_ge(l_nt[t % 2], 16 * ((t - t % 2) // 2))
                        a.wait_ge(do[t % 2], 16 * ((t - t % 2) // 2))
                    a.activation(SNT[p][0:64, :], pspr[0:64, 0:64], TANH).then_inc(act_s, 1)

            # ================= VECTOR (DVE) =================
            @block.vector
            def _(v):
                MUL = mybir.AluOpType.mult
                ADD = mybir.AluOpType.add

                v.wait_ge(gi, N_MEMSET)
                # prologue: LATC copy from psd0[0]
                v.wait_ge(lat_s, 1)
                v.tensor_copy(LATC[:, :], psd0[0][:, 0:HGS]).then_inc(lat_s, 1)

                def layer_chain(bm_done, sig_done, tanh_done, tr_done, l_sem, l_val,
                                S, GC, TMP, TTt, HTt, pstr, HSTGt, dh_wait=None):
                    # tmp_i | tmp_f = [sig_i|sig_f] * [tanh_g|c]
                    v.wait_ge(act_s, sig_done)
                    if dh_wait is not None:
                        v.wait_ge(dgc, dh_wait)
                    v.tensor_tensor(TMP[:, :], S[:, 0:128], GC[:, 0:128], MUL).then_inc(dve_s, 1)
                    # c_new (same-engine RAW on TMP needs the sem edge)
                    v.wait_ge(dve_s, bm_done)
                    v.tensor_tensor(GC[:, 64:128], TMP[:, 0:64], TMP[:, 64:128], ADD).then_inc(dve_s, 1)
                    # h = sig_o * tanh(c)
                    v.wait_ge(act_s, tanh_done)
                    v.tensor_tensor(HTt[:, :], S[:, 128:192], TTt[:, :], MUL).then_inc(dve_s, 1)
                    # copy transpose psum -> staging
                    v.wait_ge(pe_s, tr_done)
                    if l_val > 0:
                        v.wait_ge(l_sem, l_val)
                    v.tensor_copy(HSTGt[:, :],
                                  pstr[:, :].bitcast(BF16)[:, 0:64]).then_inc(dve_s, 1)

                for ct in range(CT):
                    p = ct % 2
                    layer_chain(dve_c(ct, 1), act_c(ct, 2), act_c(ct, 3), pe_c(ct, 2),
                                l_h0[p], 16 * snd_c(ct), S0, GCC0, TMP0, TT0, HT0,
                                pstr0, HSTG0[p])
                    layer_chain(dve_c(ct, 5), act_c(ct, 5), act_c(ct, 6), pe_c(ct, 4),
                                l_h1[p], 16 * snd_c(ct), S1, GCC1, TMP1, TT1, HT1,
                                pstr1, HSTG1[p])
                    # copy E psum -> EBUF (DRAM-bounced by sync)
                    v.wait_ge(pe_s, pe_c(ct, 6))
                    if (ct - ct % 2) // 2 > 0:
                        v.wait_ge(de[ct % 2], 16 * ((ct - ct % 2) // 2))
                    v.tensor_copy(EBUF[:, HGS * (ct % 2):HGS * (ct % 2 + 1)],
                                  psem[:, 0:HGS]).then_inc(dve_s, 1)
                # decoder
                for t in range(T):
                    p = t % 2
                    s, n = divmod(t, nnotes)
                    dhw = 32 * (s + 1) if n == 0 else None
                    layer_chain(dve_d(t, 1), act_d(t, 2), act_d(t, 3), pe_d(t, 2),
                                l_h0[p], 16 * snd_d(t), S0, GCD0, TMP0, TT0, HT0,
                                pstr0, HSTG0[p], dh_wait=dhw)
                    layer_chain(dve_d(t, 5), act_d(t, 5), act_d(t, 6), pe_d(t, 4),
                                l_h1[p], 16 * snd_d(t), S1, GCD1, TMP1, TT1, HT1,
                                pstr1, HSTG1[p])

    nc.compile()
    return nc


# ======================= host-side preparation =======================

def _gate_cols(core):
    """Column indices (into 4H gate dim, PyTorch i,f,g,o order) for one
    core's 512-gate slice, packed as two halves: A = [i|f|o|g] x lo-64,
    B = [i|f|o|g] x hi-64."""
    ix = []
    for half in range(2):
        base = core * 128 + half * 64
        for goff in (0, H, 3 * H, 2 * H):   # i, f, o, g
            ix.extend(range(goff + base, goff + base + 64))
    return np.array(ix)


def _bf16(x):
    import ml_dtypes
    return np.asarray(x, np.float32).astype(ml_dtypes.bfloat16)


def prep_inputs(inputs, nsub=16, nnotes=32):
    f = lambda x: np.asarray(x, dtype=np.float32)
    latent = f(inputs["latent"])
    h0_dec = f(inputs["h0_dec"])[:nsub]
    c0_dec = f(inputs["c0_dec"])[:nsub]

    def pack_k(wT, kt):
        K, N = wT.shape
        assert K == kt * 128
        out = np.empty((128, kt * N), np.float32)
        for k in range(kt):
            out[:, N * k:N * (k + 1)] = wT[128 * k:128 * (k + 1), :]
        return out

    def pack_k64(wT, kt):
        K, N = wT.shape
        assert K == kt * 64
        out = np.empty((64, kt * N), np.float32)
        for k in range(kt):
            out[:, N * k:N * (k + 1)] = wT[64 * k:64 * (k + 1), :]
        return out

    # h0T packed: [s, 128, (l k b)]
    h0T = np.einsum("slbk->slkb", h0_dec)  # [s, l, 1024, 64]
    h0T_packed = np.empty((nsub, 128, 2 * KT_H * SL), np.float32)
    for s in range(nsub):
        for l in range(2):
            for k in range(KT_H):
                h0T_packed[s, :, (l * KT_H + k) * SL:(l * KT_H + k + 1) * SL] = \
                    h0T[s, l, 128 * k:128 * (k + 1), :]

    latT = np.ascontiguousarray(latent.T)  # [512, 64]
    latT_packed = pack_k(latT, KT_L)

    ident64 = np.eye(64, dtype=np.float32)
    identT = np.eye(128, dtype=np.float32)
    ones_row = np.ones((1, SL), np.float32)

    Wih_d0, Whh_d0 = f(inputs["Wih_d0"]), f(inputs["Whh_d0"])
    Wih_d1, Whh_d1 = f(inputs["Wih_d1"]), f(inputs["Whh_d1"])
    Wdo, bdo = f(inputs["Wdo"]), f(inputs["bdo"])
    Wih_c0, Whh_c0 = f(inputs["Wih_c0"]), f(inputs["Whh_c0"])
    Wih_c1, Whh_c1 = f(inputs["Wih_c1"]), f(inputs["Whh_c1"])
    Wco, bco = f(inputs["Wco"]), f(inputs["bco"])
    b0_full = f(inputs["bih_d0"]) + f(inputs["bhh_d0"])
    b1_full = f(inputs["bih_d1"]) + f(inputs["bhh_d1"])
    bc0_full = f(inputs["bih_c0"]) + f(inputs["bhh_c0"])
    bc1_full = f(inputs["bih_c1"]) + f(inputs["bhh_c1"])

    Wdo_pad = np.zeros((INPUT_PAD, H), np.float32)
    Wdo_pad[:INPUT] = Wdo
    bdo_pad = np.zeros(INPUT_PAD, np.float32)
    bdo_pad[:INPUT] = bdo

    COND_OUT = 512
    in_maps = []
    for core in range(NC):
        ix = _gate_cols(core)

        def slc(w):
            # w: [4H, K] -> [K, 512] slice in halves order
            return np.ascontiguousarray(w[ix, :].T.astype(np.float32))

        wx0_full = np.zeros((INPUT_PAD, GSL), np.float32)
        wx0_full[:INPUT] = slc(Wih_d0[:, :INPUT])
        wemb_full = slc(Wih_d0[:, INPUT:INPUT + COND_OUT])  # [512, 512]

        # c0s interleaved layout: [s*2+l, 128, 64]
        c0i = np.empty((nsub * 2, 128, SL), np.float32)
        for s in range(nsub):
            for l in range(2):
                blk = c0_dec[s, l, :, core * 128:(core + 1) * 128]  # [64, 128]
                c0i[s * 2 + l, 0:64, :] = blk[:, 0:64]
                c0i[s * 2 + l, 64:128, :] = blk[:, 64:128]

        m = {
            "latT": latT_packed,
            "h0T": h0T_packed,
            "c0s": c0i,
            "wx0": pack_k64(wx0_full, 8)[:, :KT_X * GSL],
            "wh0": pack_k(slc(Whh_d0), KT_H),
            "wx1": pack_k(slc(Wih_d1), KT_H),
            "wh1": pack_k(slc(Whh_d1), KT_H),
            "wdoT": pack_k(np.ascontiguousarray(Wdo_pad.T[:, core * SL:(core + 1) * SL]), KT_H),
            "wemb": pack_k64(wemb_full, 8),
            "wxc0": pack_k(slc(Wih_c0), KT_L),
            "whc0": pack_k(slc(Whh_c0), KT_H),
            "wxc1": pack_k(slc(Wih_c1), KT_H),
            "whc1": pack_k(slc(Whh_c1), KT_H),
            "wcoT": pack_k(np.ascontiguousarray(Wco.T[:, core * SL:(core + 1) * SL]), KT_H),
            "b1r": b1_full[ix][None, :],
            "bdoc": bdo_pad[core * SL:(core + 1) * SL][None, :],
            "b0r": b0_full[ix][None, :],
            "bc0r": bc0_full[ix][None, :],
            "bc1r": bc1_full[ix][None, :],
            "bcoc": bco[core * SL:(core + 1) * SL][None, :],
            "onesr": ones_row,
            "id64": ident64,
            "idT": identT,
        }
        mm = {k: (_bf16(v) if k != "c0s" else np.ascontiguousarray(v, np.float32))
              for k, v in m.items()}
        in_maps.append(mm)
    return in_maps


def assemble_output(results, nsub=16, nnotes=32):
    T = nsub * nnotes
    # each core: out [T, 64, 32] f32 holding bf16 pairs -> decode
    outs = []
    for c in range(NC):
        raw = np.ascontiguousarray(np.asarray(results[c]["out"], np.float32))
        u16 = raw.view(np.uint16).reshape(T, 64, 64)
        f32 = (u16.astype(np.uint32) << 16).view(np.float32)
        outs.append(f32)
    full = np.concatenate(outs, axis=1)  # [T, 512, 64]
    return np.ascontiguousarray(full[:, :INPUT, :].transpose(2, 0, 1))


_CACHED = {}


def kernel(**inputs) -> np.ndarray:
    from concourse.bass_utils import run_bass_kernel_spmd
    nsub, nnotes = 16, 32
    key = (nsub, nnotes)
    if key not in _CACHED:
        _CACHED[key] = build(nsub, nnotes)
    nc = _CACHED[key]
    in_maps = prep_inputs(inputs, nsub, nnotes)
    res = run_bass_kernel_spmd(nc, in_maps, core_ids=list(range(NC)))
    return assemble_output(res.results, nsub, nnotes)
